# revision 10
# baseline (speedup 1.0000x reference)
"""CTC loss kernel for Trainium2 (Bass/Tile), 8-core data-parallel over batch.

v2: forward+backward split. Each core runs TWO independent 512-step CTC
DP chains (forward from t=0, backward from t=T in reversed-s coordinates,
which makes the backward recurrence structurally identical to forward).
The chains' DVE ops are interleaved instruction-by-instruction so each
op's semaphore wait (~220ns completion latency) is hidden behind the
other chain's op. Loss combines on host: P = sum_s alpha_512[s]*beta_512[s].

Other changes vs v1:
  - logits shipped as bf16; gather matmuls + transposes run bf16 (1-pass
    PE instead of fp32's 2-pass LOW/HIGH).
  - renorm (every 32 steps/chain) is pure-DVE integer-exponent math:
    scale = 2^-E extracted via bitcast/shift, logacc accumulates biased
    exponents; cross-quadrant sync factors fc/fd built with int ALU ops.
    No ACT Ln/Exp in the DP path -> no activation-table thrashing.
  - lse: per-chunk ln batched into one [128,32] Ln at the end.
  - halo evacuation on DVE (was ACT).
  - phase_a (gather) PE bursts emitted in blocks between DP groups so
    halo/renorm PE matmuls don't queue behind 26-deep gather bursts.
"""

import os
import sys
from itertools import zip_longest

import numpy as np

sys.path.insert(0, "/opt/trn_rl_repo")

# ---- problem constants (hardcoded per contract) ----
B, T, V, L = 32, 1024, 1024, 100
S = 2 * L + 1  # 201
BLANK = V - 1
N_CORES = 8
BPC = B // N_CORES  # 4

# ---- DP layout constants ----
NQ = 4          # S-chunks (one per SBUF quadrant)
CH = 51         # owned states per chunk
K = 16          # steps per halo group
H = 2 * K       # left halo width (32)
W = CH + H      # window cols per chunk (83)
KR = 32         # renorm period (steps)
TC = 128        # t-chunk size for gather phase
TH = T // 2     # steps per chain (512)
QGC = 32        # steps per q-load chunk
NVT = V // 128  # 8
SPAD = 208
ST1 = S - 128   # 73
NGH = TH // K   # 32 groups per chain
R_EV = len([g for g in range(1, NGH) if (g * K) % KR == 0])  # 15 renorms/chain

# staging scatter pieces: (partition_lo, count, row_base(qd), w0) per s-tile
PIECES = {
    0: [(0, 51, 0, 32), (19, 83, 1, 0), (70, 58, 2, 0), (121, 7, 3, 0)],
    1: [(0, 25, 2, 58), (0, 73, 3, 7)],
}


def _build_bass():
    import concourse.bacc as bacc
    import concourse.bass as bass
    import concourse.tile as tile
    from concourse import mybir

    f32 = mybir.dt.float32
    bf16 = mybir.dt.bfloat16
    i32 = mybir.dt.int32
    AF = mybir.ActivationFunctionType
    OP = mybir.AluOpType
    AX = mybir.AxisListType

    nc = bacc.Bacc("TRN2", target_bir_lowering=False)

    logits_in = nc.dram_tensor("logits", [BPC, T, V], bf16, kind="ExternalInput")
    eoh_in = [
        nc.dram_tensor(f"eoh{c}", [BPC, NVT, 128, SPAD], bf16, kind="ExternalInput")
        for c in range(2)
    ]
    mdp_in = [
        nc.dram_tensor(f"mdp{c}", [128, W], f32, kind="ExternalInput")
        for c in range(2)
    ]
    ident_in = nc.dram_tensor("ident", [128, 128], bf16, kind="ExternalInput")
    pdown_in = nc.dram_tensor("pdown", [128, 128], f32, kind="ExternalInput")
    out_a = [
        nc.dram_tensor(f"out_a{c}", [128, 2 * W], f32, kind="ExternalOutput")
        for c in range(2)
    ]
    out_lacc = nc.dram_tensor("out_lacc", [128, 2], f32, kind="ExternalOutput")
    out_lnses = nc.dram_tensor("out_lnses", [128, 32], f32, kind="ExternalOutput")

    with tile.TileContext(nc) as tc_:
        import contextlib

        with contextlib.ExitStack() as ctx:
            singles = ctx.enter_context(tc_.tile_pool(name="singles", bufs=1))
            lgp = ctx.enter_context(tc_.tile_pool(name="lgp", bufs=3))
            scrp = ctx.enter_context(tc_.tile_pool(name="scrp", bufs=2))
            ptp = ctx.enter_context(tc_.tile_pool(name="ptp", bufs=2, space="PSUM"))
            pgp = ctx.enter_context(tc_.tile_pool(name="pgp", bufs=2, space="PSUM"))
            php = ctx.enter_context(tc_.tile_pool(name="php", bufs=2, space="PSUM"))
            lgtp = ctx.enter_context(tc_.tile_pool(name="lgtp", bufs=2))
            qsbp = ctx.enter_context(tc_.tile_pool(name="qsbp", bufs=3))
            dramp = ctx.enter_context(tc_.tile_pool(name="dramp", bufs=1, space="DRAM"))

            WT = W * TH

            # ---- persistent SBUF ----
            ident_t = singles.tile([128, 128], bf16, tag="ident")
            nc.sync.dma_start(ident_t[:], ident_in[:])
            pdown_t = singles.tile([128, 128], f32, tag="pdown")
            nc.sync.dma_start(pdown_t[:], pdown_in[:])

            zeros_t = singles.tile([128, TH], f32, tag="zeros")
            nc.vector.memset(zeros_t[:], 0.0)

            ses_t = singles.tile([128, 32], f32, tag="ses")
            lnses_t = singles.tile([128, 32], f32, tag="lnses")

            M23 = float(1 << 23)  # 2^23, for building float bit patterns

            # ---- per-chain state ----
            chains = []
            for cn in range(2):
                C = {}
                C["qdp"] = dramp.tile([128, WT], f32, name=f"qdp{cn}", tag=f"qdp{cn}")
                C["e"] = [
                    singles.tile(
                        [128, NVT * SPAD], bf16, name=f"e{cn}_{b}", tag=f"e{cn}_{b}"
                    )
                    for b in range(BPC)
                ]
                for b in range(BPC):
                    for vt in range(NVT):
                        nc.sync.dma_start(
                            C["e"][b][:, vt * SPAD : (vt + 1) * SPAD],
                            eoh_in[cn][b, vt],
                        )
                C["mdp"] = singles.tile([128, W], f32, name=f"mdp{cn}", tag=f"mdp{cn}")
                nc.sync.dma_start(C["mdp"][:], mdp_in[cn][:])
                C["qg"] = [
                    singles.tile(
                        [128, 2 * W * QGC], f32, name=f"qg{cn}_{i}", tag=f"qg{cn}_{i}"
                    )
                    for i in range(2)
                ]
                for i in range(2):
                    nc.vector.memset(C["qg"][i][:], 0.0)
                # zero staging pads
                for w in range(H):
                    nc.sync.dma_start(
                        C["qdp"][0:BPC, w * TH : w * TH + TH], zeros_t[0:BPC, :]
                    )
                for w in range(S - 3 * CH + H, W):
                    nc.sync.dma_start(
                        C["qdp"][96 : 96 + BPC, w * TH : w * TH + TH],
                        zeros_t[0:BPC, :],
                    )
                C["alpha"] = singles.tile([128, 2 * W], f32, name=f"alpha{cn}", tag=f"alpha{cn}")
                C["u"] = singles.tile([128, W], f32, name=f"u{cn}", tag=f"u{cn}")
                C["v"] = singles.tile([128, W], f32, name=f"v{cn}", tag=f"v{cn}")
                nc.vector.memset(C["alpha"][:], 0.0)
                nc.vector.memset(C["u"][:], 0.0)
                nc.vector.memset(C["v"][:], 0.0)
                nc.vector.memset(C["alpha"][0:BPC, H : H + 1], 1.0)
                C["lacc"] = singles.tile([128, 1], f32, name=f"lacc{cn}", tag=f"lacc{cn}")
                nc.vector.memset(C["lacc"][:], 0.0)
                # renorm temps
                for nm in ("m1", "d", "bitsf", "Eaf", "Ef", "lsh", "dl2",
                           "dlp2", "dlm2", "fcf", "tf", "t2", "t3"):
                    C[nm] = singles.tile([128, 1], f32, name=f"{nm}{cn}", tag=f"{nm}{cn}")
                for nm in ("Ei", "fcb", "faci"):
                    C[nm] = singles.tile([128, 1], i32, name=f"{nm}{cn}", tag=f"{nm}{cn}")
                nc.vector.memset(C["fcb"][:], 127 << 23)  # fc = 1.0
                chains.append(C)

            # ---------- gather + lse phase ----------
            def phase_a_block(itc, b):
                """Gather+lse for (t-chunk itc, batch b). itc<4 -> fw, else bw."""
                C = chains[0] if itc < 4 else chains[1]
                col0 = itc * TC if itc < 4 else itc * TC - TH
                lg = lgp.tile([128, V], bf16, tag="lg")
                nc.sync.dma_start(lg[:], logits_in[b, itc * TC : (itc + 1) * TC, :])
                scr = scrp.tile([128, V], bf16, tag="scr")
                col = itc * 4 + b
                nc.scalar.activation(
                    scr[:], lg[:], AF.Exp, accum_out=ses_t[:, col : col + 1]
                )
                lgt = lgtp.tile([128, NVT * 128], bf16, tag="lgt")
                for vt in range(NVT):
                    pt = ptp.tile([128, 128], bf16, tag="pt")
                    nc.tensor.transpose(
                        pt[:], lg[:, vt * 128 : (vt + 1) * 128], ident_t[:]
                    )
                    nc.scalar.copy(lgt[:, vt * 128 : (vt + 1) * 128], pt[:])
                for st in range(2):
                    srows = 128 if st == 0 else ST1
                    pg = pgp.tile([128, 128], f32, tag="pg")
                    for vt in range(NVT):
                        nc.tensor.matmul(
                            pg[0:srows, :],
                            C["e"][b][
                                :, vt * SPAD + st * 128 : vt * SPAD + st * 128 + srows
                            ],
                            lgt[:, vt * 128 : (vt + 1) * 128],
                            start=(vt == 0),
                            stop=(vt == NVT - 1),
                        )
                    qsb = qsbp.tile([128, 128], f32, tag="qsb")
                    nc.scalar.activation(qsb[0:srows, :], pg[0:srows, :], AF.Exp)
                    for (plo, cnt, qd, w0) in PIECES[st]:
                        row = 32 * qd + b
                        dst = bass.AP(
                            tensor=C["qdp"].tensor,
                            offset=C["qdp"][row : row + 1, w0 * TH + col0].offset,
                            ap=[[TH, cnt], [1, TC]],
                        )
                        nc.sync.dma_start(dst, qsb[plo : plo + cnt, :])

            def load_qchunk(C, jc, buf):
                qg = C["qg"][buf % 2]
                rl = qg.ap[0][0]
                for qd in range(NQ):
                    src_ap = bass.AP(
                        tensor=C["qdp"].tensor,
                        offset=C["qdp"][32 * qd : 32 * qd + 1, jc * QGC : jc * QGC + 1].offset,
                        ap=[[WT, BPC], [TH, W], [1, QGC]],
                    )
                    dst_ap = bass.AP(
                        tensor=qg.tensor,
                        offset=qg[32 * qd : 32 * qd + 1, 0:1].offset,
                        ap=[[rl, BPC], [QGC, W], [1, QGC]],
                    )
                    nc.sync.dma_start(dst_ap, src_ap)
                q_half = bass.AP(
                    tensor=qg.tensor,
                    offset=qg[0:128, 0:1].offset,
                    ap=[qg.ap[0], [QGC, W], [1, QGC]],
                )
                qm_half = bass.AP(
                    tensor=qg.tensor,
                    offset=qg[0:128, W * QGC : W * QGC + 1].offset,
                    ap=[qg.ap[0], [QGC, W], [1, QGC]],
                )
                m_b = bass.AP(
                    tensor=C["mdp"].tensor,
                    offset=C["mdp"][0:128, 0:1].offset,
                    ap=[C["mdp"].ap[0], [1, W], [0, QGC]],
                )
                nc.gpsimd.tensor_tensor(qm_half, q_half, m_b, OP.mult)

            # ---------- DP ops (as thunk lists for fw/bw interleave) ----------
            def renorm_ops(C):
                al, lacc = C["alpha"], C["lacc"]
                ps = php.tile([128, 1], f32, tag="psr")
                return [
                    lambda: nc.vector.tensor_reduce(
                        C["m1"][:], al[:, 0:W], AX.X, OP.max
                    ),
                    lambda: nc.vector.tensor_single_scalar(
                        C["d"][:], C["m1"][:], 1e-30, OP.max
                    ),
                    # biased exponent E of d: float(bitcast_i32(d)) * 2^-23,
                    # minus 0.5 so either rounding mode lands within +-1 of
                    # floor (any consistent power-of-2 scale is correct).
                    lambda: nc.vector.tensor_copy(
                        C["bitsf"][:], C["d"][:].bitcast(i32)
                    ),
                    lambda: nc.vector.tensor_scalar(
                        C["Eaf"][:], C["bitsf"][:], 1.0 / M23, -0.5,
                        OP.mult, OP.add,
                    ),
                    lambda: nc.vector.tensor_copy(C["Ei"][:], C["Eaf"][:]),
                    lambda: nc.vector.tensor_copy(C["Ef"][:], C["Ei"][:]),
                    lambda: nc.vector.tensor_add(lacc[:], lacc[:], C["Ef"][:]),
                    lambda: nc.tensor.matmul(ps[:], pdown_t[:], lacc[:]),
                    lambda: nc.vector.tensor_copy(C["lsh"][:], ps[:]),
                    lambda: nc.vector.tensor_sub(C["dl2"][:], C["lsh"][:], lacc[:]),
                    lambda: nc.vector.tensor_scalar_max(C["dlp2"][:], C["dl2"][:], 0.0),
                    lambda: nc.vector.memset(C["dlp2"][0:32, :], 0.0),
                    lambda: nc.vector.tensor_scalar(
                        C["dlm2"][:], C["dl2"][:], 0.0, -126.0, OP.min, OP.max
                    ),
                    lambda: nc.vector.memset(C["dlm2"][0:32, :], 0.0),
                    # fc = 2^dlm2: bits = (dlm2 + 127) * 2^23 (exact in f32)
                    lambda: nc.vector.tensor_scalar(
                        C["fcf"][:], C["dlm2"][:], 127.0, M23, OP.add, OP.mult
                    ),
                    lambda: nc.vector.tensor_copy(C["fcb"][:], C["fcf"][:]),
                    # fac = 2^(127 - E - dlp2), clamped to >= 2^-126
                    lambda: nc.vector.tensor_add(C["tf"][:], C["Ef"][:], C["dlp2"][:]),
                    lambda: nc.vector.tensor_scalar(
                        C["t2"][:], C["tf"][:], -1.0, 254.0, OP.mult, OP.add
                    ),
                    lambda: nc.vector.tensor_scalar(
                        C["t3"][:], C["t2"][:], 1.0, M23, OP.max, OP.mult
                    ),
                    lambda: nc.vector.tensor_copy(C["faci"][:], C["t3"][:]),
                    lambda: nc.vector.tensor_add(lacc[:], lacc[:], C["dlp2"][:]),
                    lambda: nc.vector.tensor_scalar_mul(
                        al[:, :], al[:, :], C["faci"][:].bitcast(f32)
                    ),
                ]

            def halo_ops(C):
                al = C["alpha"]
                psh = php.tile([128, 2 * H], f32, tag="psh")
                h_src = bass.AP(
                    tensor=al.tensor,
                    offset=al[0:128, CH : CH + 1].offset,
                    ap=[al.ap[0], [W, 2], [1, H]],
                )
                h_dst = bass.AP(
                    tensor=al.tensor,
                    offset=al[0:128, 0:1].offset,
                    ap=[al.ap[0], [W, 2], [1, H]],
                )

                def mm():
                    nc.tensor.matmul(psh[:], pdown_t[:], h_src)

                def evac():
                    h_in = bass.AP(
                        tensor=psh.tensor,
                        offset=psh[0:128, 0:1].offset,
                        ap=[psh.ap[0], [H, 2], [1, H]],
                    )
                    nc.vector.tensor_scalar_mul(
                        h_dst, h_in, C["fcb"][:].bitcast(f32)
                    )

                return [mm, evac]

            def step_ops(C, tau, buf):
                al, u, v = C["alpha"], C["u"], C["v"]
                qg = C["qg"][buf % 2]

                def op1():
                    nc.vector.scalar_tensor_tensor(
                        u[:, 2:W], al[:, 1 : W - 1], 0.0, al[:, 2:W], OP.add, OP.add
                    )

                def op2():
                    nc.vector.scalar_tensor_tensor(
                        v[:, 2:W], u[:, 2:W], 0.0, al[:, W : 2 * W - 2],
                        OP.add, OP.add,
                    )

                def op3():
                    out_ap = bass.AP(
                        tensor=al.tensor,
                        offset=al[0:128, 2:3].offset,
                        ap=[al.ap[0], [W, 2], [1, W - 2]],
                    )
                    v_dup = bass.AP(
                        tensor=v.tensor,
                        offset=v[0:128, 2:3].offset,
                        ap=[v.ap[0], [0, 2], [1, W - 2]],
                    )
                    q_ap = bass.AP(
                        tensor=qg.tensor,
                        offset=qg[0:128, 2 * QGC + tau : 2 * QGC + tau + 1].offset,
                        ap=[qg.ap[0], [W * QGC, 2], [QGC, W - 2]],
                    )
                    nc.vector.scalar_tensor_tensor(
                        out_ap, v_dup, 1.0, q_ap, OP.mult, OP.mult
                    )

                return [op1, op2, op3]

            def group_ops(cn, g):
                C = chains[cn]
                ops = []
                if g > 0 and (g * K) % KR == 0:
                    ops += renorm_ops(C)
                if g > 0:
                    ops += halo_ops(C)
                buf = g // 2
                for j in range(K):
                    if cn == 0:
                        tau = (g % 2) * K + j
                    else:
                        tau = (QGC - 1 if g % 2 == 0 else K - 1) - j
                    ops += step_ops(C, tau, buf)
                return ops

            # ---------- main schedule ----------
            # gather runs a full pair ahead of the DP (pair p+2 staged during
            # pair p's groups) so chunk prefetches always have emitted staging.
            for blk in range(2 * BPC):
                phase_a_block(0 if blk % 2 == 0 else 7, blk // 2)
            load_qchunk(chains[0], 0, 0)
            load_qchunk(chains[1], 15, 0)
            for blk in range(2 * BPC):
                phase_a_block(1 if blk % 2 == 0 else 6, blk // 2)

            for ci in range(16):
                p, sub = ci // 4, ci % 4
                # prefetch next chunk (double-buffered) while ci's groups run
                if ci + 1 < 16:
                    load_qchunk(chains[0], ci + 1, ci + 1)
                    load_qchunk(chains[1], 15 - (ci + 1), ci + 1)
                for gi, g in enumerate((2 * ci, 2 * ci + 1)):
                    if p < 2:
                        blk = 2 * sub + gi
                        itc = (p + 2) if blk % 2 == 0 else (5 - p)
                        phase_a_block(itc, blk // 2)
                    fa = group_ops(0, g)
                    fb = group_ops(1, g)
                    for x, y in zip_longest(fa, fb):
                        if x is not None:
                            x()
                        if y is not None:
                            y()

            # ---------- outputs ----------
            nc.scalar.activation(lnses_t[:], ses_t[:], AF.Ln)
            for cn in range(2):
                nc.sync.dma_start(out_a[cn][:, :], chains[cn]["alpha"][:, :])
                nc.sync.dma_start(
                    out_lacc[:, cn : cn + 1], chains[cn]["lacc"][:]
                )
            nc.sync.dma_start(out_lnses[:, :], lnses_t[:])

    nc.compile()
    return nc


def _host_prep(targets_np, logits_bf, core):
    """Build per-core input map. logits_bf: full [B,T,V] bf16 array."""
    import ml_dtypes

    bf16 = ml_dtypes.bfloat16
    bs = core * BPC
    tg = targets_np[bs : bs + BPC]
    ext = np.full((BPC, S), BLANK, dtype=np.int64)
    ext[:, 1::2] = tg
    m = np.zeros((BPC, S), dtype=np.float32)
    m[:, 2:] = ((ext[:, 2:] != BLANK) & (ext[:, 2:] != ext[:, :-2])).astype(np.float32)
    # fw z-mask: mp[s] = m[s+2]; bw z-mask: mbp[x] = m[S-1-x]
    mp = np.zeros((BPC, S), dtype=np.float32)
    mp[:, : S - 2] = m[:, 2:]
    mbp = np.zeros((BPC, S), dtype=np.float32)
    mbp[:, : S - 2] = m[:, :1:-1][:, : S - 2]  # mbp[x] = m[S-1-x], x<=S-3

    eoh = np.zeros((2, BPC, NVT, 128, SPAD), dtype=bf16)
    for b in range(BPC):
        for s in range(S):
            vf = ext[b, s]
            eoh[0, b, vf // 128, vf % 128, s] = 1.0
            vb = ext[b, S - 1 - s]
            eoh[1, b, vb // 128, vb % 128, s] = 1.0

    mdp = np.zeros((2, 128, W), dtype=np.float32)
    for cn, mm_ in ((0, mp), (1, mbp)):
        for qd in range(NQ):
            for b in range(BPC):
                for w in range(W):
                    s = CH * qd - H + w
                    if 0 <= s < S:
                        mdp[cn, 32 * qd + b, w] = mm_[b, s]

    ident = np.eye(128, dtype=bf16)
    pdown = np.zeros((128, 128), dtype=np.float32)
    for q_ in range(32, 128):
        pdown[q_ - 32, q_] = 1.0
    return {
        "logits": np.ascontiguousarray(logits_bf[bs : bs + BPC]),
        "eoh0": eoh[0],
        "eoh1": eoh[1],
        "mdp0": mdp[0],
        "mdp1": mdp[1],
        "ident": ident,
        "pdown": pdown,
    }


_CACHED_NC = None
_LAST_RESULT = None


def _logsumexp(v):
    v = np.asarray(v, dtype=np.float64)
    mx = v.max()
    if not np.isfinite(mx):
        return mx
    return mx + np.log(np.exp(v - mx).sum())


def kernel(targets, logits):
    global _CACHED_NC, _LAST_RESULT
    import ml_dtypes
    from concourse.bass_utils import run_bass_kernel_spmd

    targets_np = np.asarray(targets)
    logits_bf = np.asarray(logits, dtype=np.float32).astype(ml_dtypes.bfloat16)

    if _CACHED_NC is None:
        _CACHED_NC = _build_bass()
    nc = _CACHED_NC

    in_maps = [_host_prep(targets_np, logits_bf, c) for c in range(N_CORES)]
    trace = bool(os.environ.get("CTC_TRACE"))
    res = run_bass_kernel_spmd(
        nc, in_maps, core_ids=list(range(N_CORES)), trace=trace
    )
    _LAST_RESULT = res

    LN2 = float(np.log(2.0))
    losses = []
    for c in range(N_CORES):
        r = res.results[c]
        af = np.asarray(r["out_a0"], dtype=np.float64)
        ab = np.asarray(r["out_a1"], dtype=np.float64)
        lacc = np.asarray(r["out_lacc"], dtype=np.float64)
        lnses = np.asarray(r["out_lnses"], dtype=np.float64)

        bs = c * BPC
        tg = targets_np[bs : bs + BPC]
        ext = np.full((BPC, S), BLANK, dtype=np.int64)
        ext[:, 1::2] = tg
        m = np.zeros((BPC, S))
        m[:, 2:] = (ext[:, 2:] != BLANK) & (ext[:, 2:] != ext[:, :-2])
        mp = np.zeros((BPC, S))
        mp[:, : S - 2] = m[:, 2:]

        lacf = (lacc[:, 0] - 127.0 * R_EV) * LN2
        lacb = (lacc[:, 1] - 127.0 * R_EV) * LN2
        for b in range(BPC):
            la = np.full(S, -np.inf)
            lg_ = np.full(S, -np.inf)  # log gamma_{TH+1}[s]
            for s in range(S):
                qd = s // CH
                w = H + s - CH * qd
                row = 32 * qd + b
                vv = af[row, w]
                if vv > 0:
                    la[s] = np.log(vv) + lacf[row]
                rr = S - 1 - s
                qdr = rr // CH
                wr = H + rr - CH * qdr
                rowr = 32 * qdr + b
                vb = ab[rowr, wr]
                if vb > 0:
                    lg_[s] = np.log(vb) + lacb[rowr]
            terms = []
            for s in range(S):
                for k_ in range(3):
                    if s + k_ >= S:
                        continue
                    if k_ == 2 and mp[b, s] == 0:
                        continue
                    t_ = la[s] + lg_[s + k_]
                    if np.isfinite(t_):
                        terms.append(t_)
            logP = _logsumexp(terms)
            lse_b = lnses[:, [itc * 4 + b for itc in range(8)]].sum()
            losses.append(-(logP - lse_b))
    return np.float32(np.mean(losses))
